# revision 1
# baseline (speedup 1.0000x reference)
"""Trainium2 Bass kernel for nn_AutoregressiveConvLSTM.

Data-parallel over batch: 32 images -> 8 cores x 4 images.

Layout per core: every 2D field (x channel, zi channel, h, c, gates) is stored
as (partition = H row 0..127, free = img*130 + 1 + w) with zero pad columns at
w offsets 0 and 129 of each image so that the 3 horizontal conv taps are plain
free-dim offset reads (dx in 0..2).

3x3 SAME convs run on the TensorEngine as banded matmuls: out = B.T @ rhs with
B[h, h'] = W[h-h'+1, dx, ci, co] a tridiagonal 128x128 "band" (vertical taps),
one matmul per (dx, ci) accumulating in PSUM; rhs is the plane with the
free-dim offset dx.

Recurrence avoids ACT table switches by using only tanh + exp
(exp_and_others set):  sigmoid(v) = 0.5*(tanh(v/2)+1).  h is stored doubled
(h2 = 2h = (tanh(o/2)+1)*tanh(c)) and the 0.5 is folded into the Whh/Wout
bands.

log prob: z = ((mu+b0) - x) * exp(-ls-b1); per-image Sum z^2 via ACT Square
with accum_out; Sum ls via DVE tensor_reduce; final cross-partition reduction
via a ones-vector matmul.
"""

import sys
import numpy as np

for _p in ("/opt/trn_rl_repo", "/root/.axon_site/_ro/trn_rl_repo"):
    if _p not in sys.path:
        sys.path.insert(0, _p)

import concourse.bacc as bacc
import concourse.mybir as mybir
from concourse import bass, tile
from concourse.bass_utils import run_bass_kernel_spmd

F32 = mybir.dt.float32
F32R = mybir.dt.float32r
AF = mybir.ActivationFunctionType
ALU = mybir.AluOpType

B, C, H, W = 32, 16, 128, 128
NCORES = 8
BL = B // NCORES          # images per core = 4
WP = W + 2                # padded row width = 130
LOG2PI = 1.8378770664093453

# band tensor indexing
N_IN = 3                        # conv_in: dx
N_IH = 8 * 3                    # conv_ih: co, dx
N_HH = 8 * 2 * 3                # conv_hh: co, ci, dx
N_OUT = 2 * 2 * 3               # conv_out: co, ci, dx
NBANDS = N_IN + N_IH + N_HH + N_OUT   # 87


def _band(w_col):
    """Build the 128x128 tridiagonal lhsT for one (ky tap column) of a 3-tap
    vertical conv: lhsT[h, h'] = w_col[h - h' + 1] for |h-h'| <= 1."""
    Bm = np.zeros((H, H), np.float32)
    idx = np.arange(H)
    for ky in range(3):
        hh = idx + ky - 1          # input row feeding output row idx
        m = (hh >= 0) & (hh < H)
        Bm[hh[m], idx[m]] = w_col[ky]
    return Bm


def _build_bands(Win, Wih, Whh, Wout):
    """All band matrices as one (87,128,128) array (lhsT layout)."""
    bands = np.zeros((NBANDS, H, H), np.float32)
    k = 0
    for dx in range(3):                        # conv_in (1->1)
        bands[k] = _band(Win[:, dx, 0, 0]); k += 1
    for co in range(8):                        # conv_ih (1->8)
        for dx in range(3):
            bands[k] = _band(Wih[:, dx, 0, co]); k += 1
    for co in range(8):                        # conv_hh (2->8), x0.5 (h2)
        for ci in range(2):
            for dx in range(3):
                bands[k] = _band(0.5 * Whh[:, dx, ci, co]); k += 1
    Wout_y = Wout[:, :, :2, :]                 # cond features are zero
    for co in range(2):                        # conv_out (2->2), x0.5 (h2)
        for ci in range(2):
            for dx in range(3):
                bands[k] = _band(0.5 * Wout_y[:, dx, ci, co]); k += 1
    assert k == NBANDS
    return bands


_CACHED = None


def _build_program(nsteps=None, skip_rec=False):
    import os
    if nsteps is None:
        nsteps = int(os.environ.get("KERNEL_T", C - 1))
    nc = bacc.Bacc(None, target_bir_lowering=False)

    xp_d = nc.dram_tensor("xp", [H, C * BL * WP], F32, kind="ExternalInput")
    bands_d = nc.dram_tensor("bands", [H, NBANDS * H], F32R, kind="ExternalInput")
    cols_d = nc.dram_tensor("cols", [H, 16], F32, kind="ExternalInput")
    out_d = nc.dram_tensor("out", [BL, 1], F32, kind="ExternalOutput")

    T = C - 1  # 15 recurrence steps
    TR = nsteps

    with tile.TileContext(nc) as tc:
        with (
            tc.tile_pool(name="const", bufs=1) as cpool,
            tc.tile_pool(name="state", bufs=1) as spool,
            tc.tile_pool(name="work", bufs=2) as wpool,
            tc.tile_pool(name="once", bufs=1) as opool,
            tc.tile_pool(name="psum", bufs=4, space=bass.MemorySpace.PSUM) as ppool,
        ):
            xall = cpool.tile([H, C, BL, WP], F32, tag="xall")
            bandsb = cpool.tile([H, NBANDS, H], F32R, tag="bands")
            cols = cpool.tile([H, 16], F32, tag="cols")
            ziall = cpool.tile([H, T, BL, WP], F32R, tag="ziall")
            ones = cpool.tile([H, 1], F32, tag="ones")

            hpair = spool.tile([H, 2, BL, WP], F32R, tag="hpair")
            cst = spool.tile([H, 2, BL, W], F32, tag="cst")
            sqcols = spool.tile([H, BL, C], F32, tag="sqcols")
            lscols = spool.tile([H, BL, C], F32, tag="lscols")

            # ---- load inputs ----
            nc.sync.dma_start(xall[:], xp_d[:])
            nc.sync.dma_start(bandsb[:], bands_d[:])
            nc.sync.dma_start(cols[:], cols_d[:])

            nc.gpsimd.memset(ziall[:].bitcast(F32), 0.0)
            nc.gpsimd.memset(hpair[:].bitcast(F32), 0.0)
            nc.gpsimd.memset(cst[:], 0.0)
            nc.gpsimd.memset(sqcols[:], 0.0)
            nc.gpsimd.memset(lscols[:], 0.0)
            nc.gpsimd.memset(ones[:], 1.0)

            def band(i):
                return bandsb[:, i, :]

            # ---- precompute zi_t = conv_in(x_t) + b_in for t in 0..14 ----
            for t in range(T):
                xr = wpool.tile([H, BL, WP], F32R, tag="xr")
                nc.vector.tensor_copy(xr[:], xall[:, t])
                zps = ppool.tile([H, BL, W], F32, tag="ps")
                for dx in range(3):
                    nc.tensor.matmul(
                        zps[:], band(dx), xr[:, :, dx:dx + W],
                        start=(dx == 0), stop=(dx == 2),
                    )
                # zi -> SBUF padded cols, +b_in
                nc.scalar.activation(
                    ziall[:, t, :, 1:1 + W], zps[:], AF.Identity,
                    bias=cols[:, 0:1],
                )

            # ---- channel 0 logprob: z0 = (x0 - b0) * exp(-b1) ----
            # Square(scale*x + bias) with scale=e^{-b1}, bias=-b0*e^{-b1}
            zjunk = opool.tile([H, BL, W], F32, tag="zjunk")
            for im in range(BL):
                nc.scalar.activation(
                    zjunk[:, im, :], xall[:, 0, im, 1:1 + W], AF.Square,
                    scale=cols[:, 2:3], bias=cols[:, 3:4],
                    accum_out=sqcols[:, im, C - 1:C],
                )

            # ---- recurrence ----
            for t in range(0 if skip_rec else TR):
                th = []  # tanh-gate tiles: i,g,f,o
                for g in range(4):
                    gps = ppool.tile([H, 2, BL, W], F32, tag="ps")
                    for half in range(2):
                        co = g * 2 + half
                        mms = []
                        for dx in range(3):
                            mms.append((N_IN + co * 3 + dx,
                                        ziall[:, t, :, dx:dx + W]))
                        if t > 0:
                            for ci in range(2):
                                for dx in range(3):
                                    mms.append((
                                        N_IN + N_IH + (co * 2 + ci) * 3 + dx,
                                        hpair[:, ci, :, dx:dx + W]))
                        for k, (bi, rhs) in enumerate(mms):
                            nc.tensor.matmul(
                                gps[:, half], band(bi), rhs,
                                start=(k == 0), stop=(k == len(mms) - 1),
                            )
                    tg = wpool.tile([H, 2, BL, W], F32, tag=f"th{g}")
                    # i,f,o: tanh(v/2 + bias'); g: tanh(v + bias)
                    scale = 1.0 if g == 1 else 0.5
                    for half in range(2):
                        co = g * 2 + half
                        nc.scalar.activation(
                            tg[:, half], gps[:, half], AF.Tanh,
                            scale=scale, bias=cols[:, 5 + co:6 + co],
                        )
                    th.append(tg)
                ti, tgg, tf, to = th

                u1 = wpool.tile([H, 2, BL, W], F32, tag="u1")
                u2 = wpool.tile([H, 2, BL, W], F32, tag="u2")
                nc.vector.scalar_tensor_tensor(
                    u1[:], tf[:], 1.0, cst[:], ALU.add, ALU.mult)
                nc.vector.scalar_tensor_tensor(
                    u2[:], ti[:], 1.0, tgg[:], ALU.add, ALU.mult)
                s2 = opool.tile([H, 2, BL, W], F32, tag="s2")
                nc.vector.tensor_add(s2[:], u1[:], u2[:])        # s2 = 2*c'
                nc.vector.tensor_scalar_mul(cst[:], s2[:], 0.5)  # c' state
                tcn = opool.tile([H, 2, BL, W], F32, tag="tcn")
                nc.scalar.activation(tcn[:], s2[:], AF.Tanh, scale=0.5)
                # h2 = (tanh(o/2)+1)*tanh(c), written into padded h tensor
                nc.vector.scalar_tensor_tensor(
                    hpair[:, :, :, 1:1 + W], to[:], 1.0, tcn[:],
                    ALU.add, ALU.mult)

                # conv_out -> mu (co 0), ls (co 1)
                pps = ppool.tile([H, 2, BL, W], F32, tag="ps")
                for co in range(2):
                    k = 0
                    for ci in range(2):
                        for dx in range(3):
                            nc.tensor.matmul(
                                pps[:, co],
                                band(N_IN + N_IH + N_HH + (co * 2 + ci) * 3 + dx),
                                hpair[:, ci, :, dx:dx + W],
                                start=(k == 0), stop=(k == 5),
                            )
                            k += 1

                E = opool.tile([H, BL, W], F32, tag="E")
                nc.scalar.activation(
                    E[:], pps[:, 1], AF.Exp, scale=-1.0, bias=cols[:, 1:2])
                d = opool.tile([H, BL, W], F32, tag="d")
                nc.vector.scalar_tensor_tensor(
                    d[:], pps[:, 0], cols[:, 4:5], xall[:, t + 1, :, 1:1 + W],
                    ALU.add, ALU.subtract)
                z = opool.tile([H, BL, W], F32, tag="z")
                nc.vector.tensor_mul(z[:], d[:], E[:])
                zj = opool.tile([H, BL, W], F32, tag="zjunk")
                for im in range(BL):
                    nc.scalar.activation(
                        zj[:, im, :], z[:, im, :], AF.Square,
                        accum_out=sqcols[:, im, t:t + 1])
                nc.vector.tensor_reduce(
                    lscols[:, :, t:t + 1], pps[:, 1], axis=mybir.AxisListType.X,
                    op=ALU.add)

            # ---- final reduction ----
            s_sq = opool.tile([H, BL, 1], F32, tag="ssq")
            s_ls = opool.tile([H, BL, 1], F32, tag="sls")
            nc.vector.tensor_reduce(
                s_sq[:], sqcols[:], axis=mybir.AxisListType.X, op=ALU.add)
            nc.vector.tensor_reduce(
                s_ls[:], lscols[:], axis=mybir.AxisListType.X, op=ALU.add)
            comb = opool.tile([H, BL], F32, tag="comb")
            nc.vector.scalar_tensor_tensor(
                comb[:], s_sq[:, :, 0], -0.5, s_ls[:, :, 0],
                ALU.mult, ALU.subtract)
            fps = ppool.tile([BL, 1], F32, tag="ps")
            nc.tensor.matmul(fps[:], comb[:], ones[:], start=True, stop=True)
            osb = opool.tile([BL, 1], F32, tag="osb")
            nc.vector.tensor_copy(osb[:], fps[:])
            nc.sync.dma_start(out_d[:], osb[:])

    nc.compile()
    return nc


def _get_program():
    global _CACHED
    if _CACHED is None:
        _CACHED = _build_program()
    return _CACHED


def kernel(x, Win, b_in, Wih, b_ih, Whh, b_hh, Wout, b_out):
    x = np.asarray(x, np.float32)
    Win = np.asarray(Win, np.float32)
    Wih = np.asarray(Wih, np.float32)
    Whh = np.asarray(Whh, np.float32)
    Wout = np.asarray(Wout, np.float32)
    b_in = np.asarray(b_in, np.float32)
    b_ih = np.asarray(b_ih, np.float32)
    b_hh = np.asarray(b_hh, np.float32)
    b_out = np.asarray(b_out, np.float32)

    bands = _build_bands(Win, Wih, Whh, Wout)
    bands_t = np.ascontiguousarray(
        np.transpose(bands, (1, 0, 2))).reshape(H, NBANDS * H)
    bt = bands_t.view(np.uint32)
    bt += 0x1000
    bt &= np.uint32(0xFFFFE000)

    # per-partition constant columns
    cols = np.zeros((H, 16), np.float32)
    b0, b1 = float(b_out[0]), float(b_out[1])
    cols[:, 0] = float(b_in[0])
    cols[:, 1] = -b1                       # exp bias: exp(-ls - b1)
    cols[:, 2] = np.exp(-b1)               # ch0 scale
    cols[:, 3] = -b0 * np.exp(-b1)         # ch0 bias
    cols[:, 4] = b0                        # d scalar
    gb = b_ih + b_hh                       # per-co gate bias, co=[i0,i1,g0,g1,f0,f1,o0,o1]
    for co in range(8):
        g = co // 2
        if g == 1:                         # g gate: tanh(v + b)
            cols[:, 5 + co] = gb[co]
        elif g == 2:                       # f gate: tanh((v + b + 1)/2)
            cols[:, 5 + co] = 0.5 * (gb[co] + 1.0)
        else:                              # i,o: tanh((v + b)/2)
            cols[:, 5 + co] = 0.5 * gb[co]

    # padded x planes per core: (C, H, BL*WP)
    in_maps = []
    for k in range(NCORES):
        xs = x[k * BL:(k + 1) * BL]        # (BL, C, H, W)
        xpad = np.zeros((C, H, BL, WP), np.float32)
        xpad[:, :, :, 1:1 + W] = np.transpose(xs, (1, 2, 0, 3))
        in_maps.append({
            "xp": np.ascontiguousarray(
                np.transpose(xpad, (1, 0, 2, 3))).reshape(H, C * BL * WP),
            "bands": bands_t,
            "cols": cols,
        })

    nc = _get_program()
    global _last_in_maps
    _last_in_maps = in_maps
    res = run_bass_kernel_spmd(nc, in_maps, core_ids=list(range(NCORES)))

    # assemble: add host-side constants
    const = -0.5 * LOG2PI * (H * W * C) - H * W * b1   # ch0 ls sum = H*W*b1
    out = np.zeros((B,), np.float32)
    for k in range(NCORES):
        out[k * BL:(k + 1) * BL] = res.results[k]["out"].reshape(BL) + const
    return out



# revision 8
# speedup vs baseline: 4.4683x; 4.4683x over previous
"""Trainium2 Bass kernel for nn_AutoregressiveConvLSTM.

Data-parallel over batch: 32 images -> 8 cores x 4 images; on-core the 4
images are split into 2 groups of 2 whose recurrences interleave so the
TensorEngine always has independent work queued (p-state friendly).

All convolutions run as fp8e4m3 DoubleRow band matmuls: a 3x3 SAME conv is
3 vertical-band (tridiagonal 128x128) lhsT matmuls at free-dim offsets
dx in 0..2; DoubleRow contracts TWO (band, plane) pairs per pass at 0.5
cycles/row.  Pairings:
  conv_hh / conv_out : (ci=0, ci=1) per dx           [hpair plane pair]
  conv_ih            : (dx0, dx1-dup) + (dx2, ones)  [bias via ones-plane]
  conv_in            : (dx0, dx1-dup) + (dx2, 0)
The dx pairs need a duplicated plane (x8 duplicated host-side, zi
duplicated by one DVE copy per step) because overlapping stride-1 pair
dims abort at runtime on HW.

Cell math is all-tanh (one ACT table): sigmoid(v) = (tanh(v/2)+1)/2,
h stored doubled (h2 = (tanh(o/2)+1)*tanh(c)), c stored doubled
(ch2 = 2c; ch2' = 0.5*(tf+1)*ch2 + (ti+1)*tg), the g-gate band is
doubled so every gate uses ACT scale 0.5, and the flax forget +1 rides
the ones-plane.  Pointwise ops run in bf16 (DVE 2x mode); z = d*E on the
otherwise-idle Pool engine.

conv_out(t) matmuls are deferred to step t+1's PE stream so the PE never
waits on the cell chain mid-step.
"""

import sys
import numpy as np

for _p in ("/opt/trn_rl_repo", "/root/.axon_site/_ro/trn_rl_repo"):
    if _p not in sys.path:
        sys.path.insert(0, _p)

import ml_dtypes
import concourse.bacc as bacc
import concourse.mybir as mybir
from concourse import bass, tile
from concourse.bass_utils import run_bass_kernel_spmd

F32 = mybir.dt.float32
BF16 = mybir.dt.bfloat16
FP8 = mybir.dt.float8e4
U8 = mybir.dt.uint8
AF = mybir.ActivationFunctionType
ALU = mybir.AluOpType
DR = mybir.MatmulPerfMode.DoubleRow
E4M3 = ml_dtypes.float8_e4m3fn

B, C, H, W = 32, 16, 128, 128
NCORES = 8
BL = B // NCORES          # images per core = 4
BG = 2                    # images per group
GS = (0, 2)               # group start image
WP = W + 2                # padded row width = 130
T = C - 1                 # recurrence steps = 15
LOG2PI = 1.8378770664093453

# pair-band indexing: [H, NPAIR, 2, H] fp8 lhsT pairs
PB_ZI = 0                                     # conv_in (dx0,dx1), (dx2, 0)
PB_IH = lambda co: 2 + co * 5                 # conv_ih (dx0,dx1), (dx2, ones)
PB_HH = lambda co, dx: 2 + co * 5 + 2 + dx    # conv_hh (ci0,ci1) per dx
PB_OUT = lambda co, dx: 42 + co * 3 + dx      # conv_out (ci0,ci1) per dx
NPAIR = 48


def _band(w_col):
    """128x128 vertical-band lhsT: B[h_in, h_out] = w_col[h_in - h_out + 1]."""
    Bm = np.zeros((H, H), np.float32)
    idx = np.arange(H)
    for ky in range(3):
        hh = idx + ky - 1
        m = (hh >= 0) & (hh < H)
        Bm[hh[m], idx[m]] = w_col[ky]
    return Bm


def _build_bands(Win, Wih, Whh, Wout, gb):
    """fp8 pair bands [NPAIR, 2, H, H] + ACT-bias residuals per gate co."""
    bands = np.zeros((NPAIR, 2, H, H), np.float32)
    bands[PB_ZI, 0] = _band(Win[:, 0, 0, 0])
    bands[PB_ZI, 1] = _band(Win[:, 1, 0, 0])
    bands[PB_ZI + 1, 0] = _band(Win[:, 2, 0, 0])

    bias_w = np.zeros(8, np.float64)     # ones-plane weight per gate co
    for co in range(8):
        g = co // 2
        s = 2.0 if g == 1 else 1.0       # g gate: tanh(v+b) = tanh((2v+2b)/2)
        extra = 1.0 if g == 2 else 0.0   # flax forget-gate +1
        bias_w[co] = s * (gb[co] + extra) / H
        bands[PB_IH(co), 0] = s * _band(Wih[:, 0, 0, co])
        bands[PB_IH(co), 1] = s * _band(Wih[:, 1, 0, co])
        bands[PB_IH(co) + 1, 0] = s * _band(Wih[:, 2, 0, co])
        bands[PB_IH(co) + 1, 1] = bias_w[co]     # dense ones-plane weight
        for dx in range(3):
            # h stored as 2h -> x0.5 ; g gate x2
            bands[PB_HH(co, dx), 0] = 0.5 * s * _band(Whh[:, dx, 0, co])
            bands[PB_HH(co, dx), 1] = 0.5 * s * _band(Whh[:, dx, 1, co])
    Wout_y = Wout[:, :, :2, :]
    for co in range(2):
        for dx in range(3):
            bands[PB_OUT(co, dx), 0] = 0.5 * _band(Wout_y[:, dx, 0, co])
            bands[PB_OUT(co, dx), 1] = 0.5 * _band(Wout_y[:, dx, 1, co])

    bands8 = bands.astype(E4M3)
    # ACT-bias residual per co (applied at tanh scale 0.5):
    resid = np.zeros(8, np.float64)
    for co in range(8):
        g = co // 2
        s = 2.0 if g == 1 else 1.0
        extra = 1.0 if g == 2 else 0.0
        want = s * (gb[co] + extra)
        got = float(bands8[PB_IH(co) + 1, 1, 0, 0]) * H
        resid[co] = 0.5 * (want - got)
    return bands8, resid


_CACHED = {}


def _pair_ap(base, stride):
    """Insert a leading free dim [stride, 2] into an AP (DoubleRow pair)."""
    dims = [list(d) for d in base.ap]
    return bass.AP(base.tensor, base.offset, [dims[0], [stride, 2]] + dims[1:])


def _flat(ap):
    """Collapse a contiguous tile AP to a single free dim (for bitcast)."""
    dims = [list(d) for d in ap.ap]
    n = 1
    for d in dims[1:]:
        n *= d[1]
    return bass.AP(ap.tensor, ap.offset, [dims[0], [1, n]])


def _build_program(act_runs):
    """act_runs: per 2-co chunk (i, g, f, o), tuple of (start, n, bias) runs."""
    import os
    TR = int(os.environ.get("KERNEL_T", T))
    nc = bacc.Bacc(None, target_bir_lowering=False)

    xbf_d = nc.dram_tensor("xbf", [H, C * BL * WP], BF16, kind="ExternalInput")
    x8_d = nc.dram_tensor("x8", [H, T * 2 * BL * WP], U8, kind="ExternalInput")
    bands_d = nc.dram_tensor("bands", [H, NPAIR * 2 * H], U8,
                             kind="ExternalInput")
    cols_d = nc.dram_tensor("cols", [H, 16], F32, kind="ExternalInput")
    ones8_d = nc.dram_tensor("ones8", [H, T * BL * WP], U8, kind="ExternalInput")
    out_d = nc.dram_tensor("out", [BL, 1], F32, kind="ExternalOutput")

    ONES8 = float(np.frombuffer(b"\x38" * 4, np.float32)[0])  # fp8 1.0 x4

    with tile.TileContext(nc) as tc:
        with (
            tc.tile_pool(name="const", bufs=1) as cpool,
            tc.tile_pool(name="state", bufs=1) as spool,
            tc.tile_pool(name="work", bufs=2) as wpool,
            tc.tile_pool(name="psum", bufs=2, space=bass.MemorySpace.PSUM) as ppool,
        ):
            xbf = cpool.tile([H, C, BL, WP], BF16, tag="xbf")
            x8 = cpool.tile([H, T, 2, BL, WP], FP8, tag="x8")
            bandsb = cpool.tile([H, NPAIR, 2, H], FP8, tag="bands")
            cols = cpool.tile([H, 16], F32, tag="cols")
            ziall = cpool.tile([H, T, 3, BL, WP], FP8, tag="ziall")

            hpair = spool.tile([H, 2, BL, WP], FP8, tag="hpair")
            ch2 = spool.tile([H, 2, BL, W], BF16, tag="ch2")
            sqcols = spool.tile([H, BL, 16], F32, tag="sqcols")
            lscols = spool.tile([H, BL, 16], F32, tag="lscols")
            ones = spool.tile([H, 1], F32, tag="ones")

            nc.sync.dma_start(bandsb[:].bitcast(U8), bands_d[:])
            nc.sync.dma_start(x8[:].bitcast(U8), x8_d[:])
            nc.sync.dma_start(cols[:], cols_d[:])
            nc.sync.dma_start(xbf[:], xbf_d[:])

            nc.gpsimd.memset(ziall[:, :, 0, :, 0:WP:WP - 1], 0.0)
            nc.sync.dma_start(ziall[:, :, 2].bitcast(U8), ones8_d[:])
            nc.gpsimd.memset(hpair[:], 0.0)
            nc.gpsimd.memset(ch2[:], 0.0)
            nc.gpsimd.memset(sqcols[:], 0.0)
            nc.gpsimd.memset(lscols[:], 0.0)
            nc.gpsimd.memset(ones[:], 1.0)

            def band(i):
                return bandsb[:, i]          # [H, 2, H]

            def x8_pair(t, gs, d0, d1):
                base = x8[:, t, 0, gs:gs + BG, d0:d0 + W]
                return _pair_ap(base, BL * WP + (d1 - d0))

            def zi_pair(t, gs, d0, slot1, d1):
                base = ziall[:, t, 0, gs:gs + BG, d0:d0 + W]
                return _pair_ap(base, slot1 * BL * WP + (d1 - d0))

            def h_pair(gs, dx):
                return hpair[:, :, gs:gs + BG, dx:dx + W]

            # ---- zi conv: x8 -> psum -> (+b_in) fp8 ziall slot0 ----
            def emit_zi(t, g):
                gs = GS[g]
                zp = ppool.tile([H, BG, W], F32, tag="zi", bufs=1)
                nc.tensor.matmul(zp[:], band(PB_ZI), x8_pair(t, gs, 0, 1),
                                 start=True, stop=False, perf_mode=DR)
                nc.tensor.matmul(zp[:], band(PB_ZI + 1), x8_pair(t, gs, 2, 0),
                                 start=False, stop=True, perf_mode=DR)
                nc.scalar.activation(ziall[:, t, 0, gs:gs + BG, 1:1 + W],
                                     zp[:], AF.Identity, bias=cols[:, 0:1])

            def emit_zi_dup(t):
                nc.gpsimd.tensor_copy(ziall[:, t, 1], ziall[:, t, 0])

            # ---- gates: 4 chunks (i, g, f, o) of 2 co each ----
            def emit_gates(t, g):
                gs = GS[g]
                chunks = []
                for ck in range(4):
                    gp = ppool.tile([H, 2, BG, W], F32, tag="g", bufs=5)
                    for cl in range(2):
                        co = ck * 2 + cl
                        mms = [(band(PB_IH(co)), zi_pair(t, gs, 0, 1, 1)),
                               (band(PB_IH(co) + 1), zi_pair(t, gs, 2, 2, 0))]
                        if t > 0:
                            for dx in range(3):
                                mms.append((band(PB_HH(co, dx)),
                                            h_pair(gs, dx)))
                        for k, (w, rhs) in enumerate(mms):
                            nc.tensor.matmul(gp[:, cl], w, rhs, start=(k == 0),
                                             stop=(k == len(mms) - 1),
                                             perf_mode=DR)
                    chunks.append(gp)
                return chunks

            def emit_cell(t, g, chunks):
                gs = GS[g]
                tgs = []
                for ck, gp in enumerate(chunks):
                    tg = wpool.tile([H, 2, BG, W], BF16, tag=f"tg{ck}_{g}")
                    for (c0, n, bv) in act_runs[ck]:
                        nc.scalar.activation(tg[:, c0:c0 + n],
                                             gp[:, c0:c0 + n], AF.Tanh,
                                             scale=0.5, bias=bv)
                    tgs.append(tg)
                ti, tgg, tf, to = tgs
                chs = ch2[:, :, gs:gs + BG, :]
                u2 = wpool.tile([H, 2, BG, W], BF16, tag=f"u2{g}")
                u1 = wpool.tile([H, 2, BG, W], BF16, tag=f"u1{g}")
                nc.vector.scalar_tensor_tensor(u2[:], ti[:], 1.0, tgg[:],
                                               ALU.add, ALU.mult)
                nc.vector.scalar_tensor_tensor(u1[:], tf[:], 1.0, chs,
                                               ALU.add, ALU.mult)
                nc.vector.scalar_tensor_tensor(chs, u1[:], 0.5, u2[:],
                                               ALU.mult, ALU.add)
                tcn = wpool.tile([H, 2, BG, W], BF16, tag=f"tcn{g}")
                nc.scalar.activation(tcn[:], chs, AF.Tanh, scale=0.5)
                for ci in range(2):
                    nc.vector.scalar_tensor_tensor(
                        hpair[:, ci, gs:gs + BG, 1:1 + W], to[:, ci], 1.0,
                        tcn[:, ci], ALU.add, ALU.mult)

            # ---- conv_out + logprob for channel t+1 (PE part) ----
            def emit_out(t, g):
                gs = GS[g]
                po = ppool.tile([H, 2, BG, W], F32, tag="o", bufs=2)
                for co in range(2):
                    for dx in range(3):
                        nc.tensor.matmul(po[:, co], band(PB_OUT(co, dx)),
                                         h_pair(gs, dx), start=(dx == 0),
                                         stop=(dx == 2), perf_mode=DR)
                return po

            def emit_logprob(t, g, po):
                gs = GS[g]
                E = wpool.tile([H, BG, W], BF16, tag=f"E{g}")
                nc.scalar.activation(E[:], po[:, 1], AF.Exp, scale=-1.0,
                                     bias=cols[:, 1:2])
                d = wpool.tile([H, BG, W], BF16, tag=f"d{g}")
                nc.vector.scalar_tensor_tensor(
                    d[:], po[:, 0], cols[:, 4:5],
                    xbf[:, t + 1, gs:gs + BG, 1:1 + W], ALU.add, ALU.subtract)
                z = wpool.tile([H, BG, W], BF16, tag=f"z{g}")
                nc.gpsimd.tensor_tensor(z[:], d[:], E[:], ALU.mult)
                zj = wpool.tile([H, BG, W], BF16, tag=f"zj{g}")
                for im in range(BG):
                    nc.vector.scalar_tensor_tensor(
                        zj[:, im], z[:, im], 1.0, z[:, im], ALU.mult, ALU.mult,
                        accum_out=sqcols[:, gs + im, t:t + 1])
                nc.vector.tensor_reduce(lscols[:, gs:gs + BG, t:t + 1],
                                        po[:, 1], axis=mybir.AxisListType.X,
                                        op=ALU.add)

            # ---- prologue ----
            for t in range(min(2, TR)):
                emit_zi(t, 0)
                emit_zi(t, 1)
                emit_zi_dup(t)

            # channel 0 logprob: z0 = (x0 - b0) * exp(-b1)
            zjunk = wpool.tile([H, BL, W], BF16, tag="zjunk")
            for im in range(BL):
                nc.scalar.activation(
                    zjunk[:, im], xbf[:, 0, im, 1:1 + W], AF.Square,
                    scale=cols[:, 2:3], bias=cols[:, 3:4],
                    accum_out=sqcols[:, im, 15:16])

            # ---- recurrence (conv_out deferred one step on the PE) ----
            for t in range(TR):
                for g in range(2):
                    if t > 0:
                        po = emit_out(t - 1, g)
                        emit_logprob(t - 1, g, po)
                    chunks = emit_gates(t, g)
                    if t + 2 < TR:
                        emit_zi(t + 2, g)
                    emit_cell(t, g, chunks)
                if t + 2 < TR:
                    emit_zi_dup(t + 2)
            for g in range(2):
                po = emit_out(TR - 1, g)
                emit_logprob(TR - 1, g, po)

            # ---- final reduction ----
            s_sq = wpool.tile([H, BL, 1], F32, tag="ssq")
            s_ls = wpool.tile([H, BL, 1], F32, tag="sls")
            nc.vector.tensor_reduce(s_sq[:], sqcols[:],
                                    axis=mybir.AxisListType.X, op=ALU.add)
            nc.vector.tensor_reduce(s_ls[:], lscols[:],
                                    axis=mybir.AxisListType.X, op=ALU.add)
            comb = wpool.tile([H, BL], F32, tag="comb")
            nc.vector.scalar_tensor_tensor(comb[:], s_sq[:, :, 0], -0.5,
                                           s_ls[:, :, 0], ALU.mult,
                                           ALU.subtract)
            fps = ppool.tile([BL, 1], F32, tag="zi", bufs=1)
            nc.tensor.matmul(fps[:], comb[:], ones[:], start=True, stop=True)
            osb = wpool.tile([BL, 1], F32, tag="osb")
            nc.vector.tensor_copy(osb[:], fps[:])
            nc.sync.dma_start(out_d[:], osb[:])

    nc.compile()
    return nc


def _get_program(act_runs):
    if act_runs not in _CACHED:
        _CACHED[act_runs] = _build_program(act_runs)
    return _CACHED[act_runs]


def kernel(x, Win, b_in, Wih, b_ih, Whh, b_hh, Wout, b_out):
    x = np.asarray(x, np.float32)
    Win = np.asarray(Win, np.float32)
    Wih = np.asarray(Wih, np.float32)
    Whh = np.asarray(Whh, np.float32)
    Wout = np.asarray(Wout, np.float32)
    b_in = np.asarray(b_in, np.float32)
    gb = (np.asarray(b_ih, np.float32) + np.asarray(b_hh, np.float32))
    b0, b1 = [float(v) for v in np.asarray(b_out, np.float32)]

    bands8, resid = _build_bands(Win, Wih, Whh, Wout, gb.astype(np.float64))

    # ACT bias runs per 2-co chunk: merge cos with equal residual bias
    def runs_for(ck):
        runs = []
        for cl in range(2):
            bv = float(resid[ck * 2 + cl])
            if abs(bv) < 1e-7:
                bv = 0.0
            if runs and runs[-1][2] == bv:
                runs[-1] = (runs[-1][0], runs[-1][1] + 1, bv)
            else:
                runs.append((cl, 1, bv))
        return tuple(runs)
    act_runs = tuple(runs_for(ck) for ck in range(4))

    cols = np.zeros((H, 16), np.float32)
    cols[:, 0] = float(b_in[0])
    cols[:, 1] = -b1
    cols[:, 2] = np.exp(-b1)
    cols[:, 3] = -b0 * np.exp(-b1)
    cols[:, 4] = b0

    bands_flat = np.ascontiguousarray(
        np.transpose(bands8, (2, 0, 1, 3))).view(np.uint8).reshape(H, -1)

    ones_u8 = np.full((H, T * BL * WP), 0x38, np.uint8)
    in_maps = []
    for k in range(NCORES):
        xs = x[k * BL:(k + 1) * BL]               # (BL, C, H, W)
        xpad = np.zeros((C, H, BL, WP), np.float32)
        xpad[:, :, :, 1:1 + W] = np.transpose(xs, (1, 2, 0, 3))
        xh = np.ascontiguousarray(np.transpose(xpad, (1, 0, 2, 3)))  # H,C,BL,WP
        x8 = xh[:, :T].astype(E4M3)               # H,T,BL,WP
        x8d = np.ascontiguousarray(np.stack([x8, x8], axis=2))  # H,T,2,BL,WP
        in_maps.append({
            "xbf": np.ascontiguousarray(xh.astype(ml_dtypes.bfloat16)
                                        ).reshape(H, -1),
            "x8": x8d.view(np.uint8).reshape(H, -1),
            "bands": bands_flat,
            "cols": cols,
            "ones8": ones_u8,
        })

    nc = _get_program(act_runs)
    global _last_in_maps, _last_nc
    _last_in_maps = in_maps
    _last_nc = nc
    res = run_bass_kernel_spmd(nc, in_maps, core_ids=list(range(NCORES)))

    const = -0.5 * LOG2PI * (H * W * C) - H * W * C * b1
    out = np.zeros((B,), np.float32)
    for k in range(NCORES):
        out[k * BL:(k + 1) * BL] = res.results[k]["out"].reshape(BL) + const
    return out
